# revision 10
# baseline (speedup 1.0000x reference)
"""DeepSeekMoE Trainium2 kernel: expert-parallel across 8 NeuronCores.

Architecture (per core, SPMD single program):
  - Router (fp32, replicated): logits = x @ rW + rb over all 4096 tokens,
    top-3 via DVE max/max_index, combine weights via ACT exp + accum.
  - 3 "expert slots" per core: 2 routed experts (core 7: 1 real + 1 dummy)
    and a 512-token slice of the shared expert.
  - Compaction (token -> compact slot) fully on-chip: prefix-sum positions
    via triangular matmul, then compact id/cw/hit tiles via factored
    one-hot matmuls (slot = pos%128 x pos//128). No indirect scatter of
    scalars (TRN2 indirect DMA uses ONE offset per partition).
  - FFN per slot (bf16 weights/activations, fp32 accum): gather x rows by
    compact ids (one 128-row indirect DMA per column), PE-transpose to
    [D, tok], mm1 + ReLU + mm2, transpose back, scale by combine weight,
    indirect scatter-ADD into fp32 partial out (unmatched slots target a
    trash row).
  - Host: sum the 8 per-core partials (expert-parallel combine).
"""
import numpy as np
import ml_dtypes

import concourse.bass as bass
import concourse.bacc as bacc
import concourse.mybir as mybir
import concourse.tile as tile
from concourse.masks import make_identity

BF16 = ml_dtypes.bfloat16

P = 128
B, L, D = 2, 2048, 1024
N = B * L            # 4096 tokens
N_PAD = N + P        # + trash rows (gather/scatter target for empty slots)
E = 15               # routed experts
H = 1024
TOPK = 3
G = N // P           # 32 token groups of 128 (token n = g*128 + p)
KD = D // P          # 8 contraction chunks over D
MH = H // P          # 8 output tiles over H
NCORES = 8
CAP = 1024           # routed-slot capacity (max fp32 count ~890 + jitter margin)
CAP_SH = 512         # shared-slot capacity (tokens per core)
CAPS = (CAP, CAP, CAP_SH)
CG = CAP // P        # 8 compact columns per routed slot
RGRP = 512           # router token-group size (fp32 moving max)

FP32 = mybir.dt.float32
BF = mybir.dt.bfloat16
I32 = mybir.dt.int32
U32 = mybir.dt.uint32
U8 = mybir.dt.uint8
AF = mybir.ActivationFunctionType
ALU = mybir.AluOpType


def build_program():
    import os
    dbg_no_ffn = bool(int(os.environ.get("MOE_NO_FFN", "0")))
    dbg_slots = os.environ.get("MOE_SLOTS", "012")
    nc = bacc.Bacc("TRN2", target_bir_lowering=False, debug=False,
                   enable_asserts=False, num_devices=NCORES)

    # ---- DRAM I/O ----
    xT_d = nc.dram_tensor("xT", [D, N], FP32, kind="ExternalInput").ap()
    xrows_d = nc.dram_tensor("xrows", [N_PAD, D], BF, kind="ExternalInput").ap()
    rW_d = nc.dram_tensor("rW", [D, E], FP32, kind="ExternalInput").ap()
    rb_d = nc.dram_tensor("rb", [E, 1], FP32, kind="ExternalInput").ap()
    w1_d = nc.dram_tensor("w1", [3, D, H], BF, kind="ExternalInput").ap()
    w2_d = nc.dram_tensor("w2", [3, H, D], BF, kind="ExternalInput").ap()
    b1_d = nc.dram_tensor("b1", [3, H], FP32, kind="ExternalInput").ap()
    b2_d = nc.dram_tensor("b2", [3, D], FP32, kind="ExternalInput").ap()
    eid_d = nc.dram_tensor("eid", [2, P], FP32, kind="ExternalInput").ap()
    shids_d = nc.dram_tensor("shids", [CAP_SH, 1], I32, kind="ExternalInput").ap()

    out_d = nc.dram_tensor("out", [N_PAD, D], FP32, kind="ExternalOutput").ap()

    with tile.TileContext(nc) as tc:
        import contextlib
        with contextlib.ExitStack() as top:
            consts = top.enter_context(tc.tile_pool(name="consts", bufs=1))
            routing = top.enter_context(tc.tile_pool(name="routing", bufs=1))

            ident_b = consts.tile([P, P], BF)
            make_identity(nc, ident_b[:])
            ident_f = consts.tile([P, P], FP32)
            make_identity(nc, ident_f[:])
            # strict upper-triangular ones (lhsT for prefix sum): [q,p]=1 iff q<p
            triu = consts.tile([P, P], BF)
            nc.gpsimd.memset(triu[:], 1.0)
            nc.gpsimd.affine_select(out=triu[:], in_=triu[:],
                                    compare_op=ALU.is_ge, fill=0.0,
                                    base=0, pattern=[[-1, P]], channel_multiplier=1)
            nc.vector.tensor_scalar(out=triu[:], in0=triu[:], scalar1=-1.0,
                                    scalar2=1.0, op0=ALU.mult, op1=ALU.add)
            ones_col = consts.tile([P, 1], BF)
            nc.vector.memset(ones_col[:], 1.0)
            ones_row = consts.tile([1, P], FP32)
            nc.vector.memset(ones_row[:], 1.0)
            # token id per (p, g): n = g*128 + p, as fp32
            ids_i = consts.tile([P, G], I32)
            nc.gpsimd.iota(ids_i[:], pattern=[[P, G]], base=0, channel_multiplier=1)
            ids_f = consts.tile([P, G], FP32)
            nc.vector.tensor_copy(out=ids_f[:], in_=ids_i[:])
            # column-index iotas (same on every partition)
            iota128_i = consts.tile([P, P], I32)
            nc.gpsimd.iota(iota128_i[:], pattern=[[1, P]], base=0, channel_multiplier=0)
            iota128 = consts.tile([P, P], FP32)
            nc.vector.tensor_copy(out=iota128[:], in_=iota128_i[:])
            iotaCG_i = consts.tile([P, CG], I32)
            nc.gpsimd.iota(iotaCG_i[:], pattern=[[1, CG]], base=0, channel_multiplier=0)
            iotaCG = consts.tile([P, CG], FP32)
            nc.vector.tensor_copy(out=iotaCG[:], in_=iotaCG_i[:])

            # routing results (live across phases)
            idx_sl = [routing.tile([P, CG], I32, name=f"idx_sl{s}") for s in range(2)]
            cw_sl = [routing.tile([P, CG], FP32, name=f"cw_sl{s}") for s in range(2)]
            sh_idx = routing.tile([P, CAP_SH // P], I32)
            nc.sync.dma_start(
                out=sh_idx[:],
                in_=shids_d.rearrange("(g p) one -> p (g one)", p=P))
            ones_cw = routing.tile([P, CG], FP32)
            nc.vector.memset(ones_cw[:], 1.0)

            # ============ Phase A: router + routing compaction ============
            with contextlib.ExitStack() as pa:
                xt = pa.enter_context(tc.tile_pool(name="xt", bufs=KD))
                rsb = pa.enter_context(tc.tile_pool(name="rsb", bufs=1))
                rscr = pa.enter_context(tc.tile_pool(name="rscr", bufs=2))
                psL = pa.enter_context(tc.tile_pool(name="psL", bufs=2, space="PSUM"))
                psT = pa.enter_context(tc.tile_pool(name="psT", bufs=2, space="PSUM"))
                psPos = pa.enter_context(tc.tile_pool(name="psPos", bufs=1, space="PSUM"))
                psC = pa.enter_context(tc.tile_pool(name="psC", bufs=1, space="PSUM"))

                xts = []
                for k in range(KD):
                    t = xt.tile([P, N], FP32, tag="xt", name=f"xt{k}")
                    nc.sync.dma_start(out=t[:], in_=xT_d[k * P:(k + 1) * P, :])
                    xts.append(t)
                rw_sb = rsb.tile([P, KD * E], FP32)
                nc.sync.dma_start(
                    out=rw_sb[:].rearrange("p (k e) -> p k e", e=E),
                    in_=rW_d.rearrange("(k p) e -> p k e", p=P))
                rb_sb = rsb.tile([E, 1], FP32)
                nc.sync.dma_start(out=rb_sb[:], in_=rb_d[:])

                logitsT = rsb.tile([P, G * E], FP32)
                ngrp = N // RGRP
                for g in range(ngrp):
                    pl = psL.tile([E, RGRP], FP32, space="PSUM", tag="psl")
                    for k in range(KD):
                        nc.tensor.matmul(
                            out=pl[:],
                            lhsT=rw_sb[:, k * E:(k + 1) * E],
                            rhs=xts[k][:, g * RGRP:(g + 1) * RGRP],
                            start=(k == 0), stop=(k == KD - 1))
                    lsb = rscr.tile([E, RGRP], FP32, tag="lsb")
                    nc.vector.tensor_scalar_add(lsb[:], pl[:], rb_sb[:, :1])
                    for t in range(RGRP // P):
                        pt = psT.tile([P, E], FP32, space="PSUM", tag="pst")
                        nc.tensor.transpose(out=pt[:], in_=lsb[:, t * P:(t + 1) * P],
                                            identity=ident_f[:E, :E])
                        gg = g * (RGRP // P) + t
                        nc.scalar.copy(out=logitsT[:, gg * E:(gg + 1) * E], in_=pt[:])

                # ---- top-3 + combine weights ----
                topi_f = rsb.tile([P, G * TOPK], FP32)
                e3 = rsb.tile([P, G * TOPK], FP32)
                den = rsb.tile([P, G], FP32)
                negm = rsb.tile([P, G], FP32)
                for g in range(G):
                    vm = rscr.tile([P, 8], FP32, tag="vm")
                    vi = rscr.tile([P, 8], U32, tag="vi")
                    lg = logitsT[:, g * E:(g + 1) * E]
                    nc.vector.max(out=vm[:], in_=lg)
                    nc.vector.max_index(out=vi[:], in_max=vm[:], in_values=lg)
                    nc.vector.tensor_copy(out=topi_f[:, g * TOPK:(g + 1) * TOPK],
                                          in_=vi[:, :TOPK])
                    nc.vector.tensor_scalar_mul(negm[:, g:g + 1], vm[:, 0:1], -1.0)
                    esc = rscr.tile([P, E], FP32, tag="esc")
                    nc.scalar.activation(esc[:], lg, AF.Exp,
                                         bias=negm[:, g:g + 1], scale=1.0,
                                         accum_out=den[:, g:g + 1])
                    nc.scalar.activation(e3[:, g * TOPK:(g + 1) * TOPK],
                                         vm[:, :TOPK], AF.Exp,
                                         bias=negm[:, g:g + 1], scale=1.0)
                rden = rsb.tile([P, G], FP32)
                nc.vector.reciprocal(rden[:], den[:])
                cw3 = rsb.tile([P, G * TOPK], FP32)
                cw3v = cw3[:].rearrange("p (g j) -> p g j", j=TOPK)
                e3v = e3[:].rearrange("p (g j) -> p g j", j=TOPK)
                for g in range(G):
                    nc.vector.tensor_scalar_mul(cw3v[:, g], e3v[:, g], rden[:, g:g + 1])

                # ---- per-routed-slot masks, positions, on-chip compaction ----
                big = rsb.tile([P, G], FP32)
                nc.vector.memset(big[:], float(1 << 20))
                for s in range(2):
                    eid = rscr.tile([P, 1], FP32, tag="eid")
                    nc.sync.dma_start(out=eid[:], in_=eid_d[s, :, None])
                    eq = [rscr.tile([P, G], FP32, tag=f"eq{j}", name=f"eq{j}")
                          for j in range(TOPK)]
                    tiv = topi_f[:].rearrange("p (g j) -> p j g", j=TOPK)
                    for j in range(TOPK):
                        nc.vector.tensor_scalar(out=eq[j][:], in0=tiv[:, j],
                                                scalar1=eid[:, :1], scalar2=None,
                                                op0=ALU.is_equal)
                    m_f = rscr.tile([P, G], FP32, tag="mf")
                    nc.vector.tensor_add(m_f[:], eq[0][:], eq[1][:])
                    nc.vector.tensor_add(m_f[:], m_f[:], eq[2][:])
                    m_bf = rscr.tile([P, G], BF, tag="mbf")
                    nc.vector.tensor_copy(out=m_bf[:], in_=m_f[:])
                    cw_s = rscr.tile([P, G], FP32, tag="cws")
                    tmp = rscr.tile([P, G], FP32, tag="cwt")
                    cwv = cw3v.rearrange("p g j -> p j g")
                    nc.vector.tensor_mul(cw_s[:], eq[0][:], cwv[:, 0])
                    nc.vector.tensor_mul(tmp[:], eq[1][:], cwv[:, 1])
                    nc.vector.tensor_add(cw_s[:], cw_s[:], tmp[:])
                    nc.vector.tensor_mul(tmp[:], eq[2][:], cwv[:, 2])
                    nc.vector.tensor_add(cw_s[:], cw_s[:], tmp[:])

                    # positions: within-group prefix (tri matmul) + group offsets
                    pp = psPos.tile([P, G], FP32, space="PSUM", tag="ppos")
                    ptot = psPos.tile([1, G], FP32, space="PSUM", tag="ptot")
                    nc.tensor.matmul(out=pp[:], lhsT=triu[:], rhs=m_bf[:],
                                     start=True, stop=False)
                    nc.tensor.matmul(out=ptot[:], lhsT=ones_col[:], rhs=m_bf[:],
                                     start=True, stop=True)
                    tot = rscr.tile([1, G], FP32, tag="tot")
                    nc.scalar.copy(out=tot[:], in_=ptot[:])
                    incl = rscr.tile([1, G], FP32, tag="incl")
                    zero1 = rscr.tile([1, G], FP32, tag="zero1")
                    nc.vector.memset(zero1[:], 0.0)
                    nc.vector.tensor_tensor_scan(out=incl[:], data0=tot[:],
                                                 data1=zero1[:], initial=0.0,
                                                 op0=ALU.add, op1=ALU.add)
                    offs = rscr.tile([1, G], FP32, tag="offs")
                    nc.vector.memset(offs[:], 0.0)
                    nc.vector.tensor_copy(out=offs[:, 1:G], in_=incl[:, 0:G - 1])
                    nc.tensor.matmul(out=pp[:], lhsT=ones_row[:], rhs=offs[:],
                                     start=False, stop=True)
                    # clamp unassigned -> BIG
                    invm = rscr.tile([P, G], U8, tag="invm")
                    nc.vector.tensor_scalar(out=invm[:], in0=m_f[:], scalar1=0.0,
                                            scalar2=None, op0=ALU.is_equal)
                    posc = rscr.tile([P, G], FP32, tag="posc")
                    nc.vector.tensor_copy(out=posc[:], in_=pp[:])
                    nc.vector.copy_predicated(out=posc[:], mask=invm[:], data=big[:])
                    # posg = pos // 128 via comparisons (exact); posp = pos - 128*posg
                    posg = rscr.tile([P, G], FP32, tag="posg")
                    nc.vector.memset(posg[:], 0.0)
                    for j in range(1, CG):
                        nc.vector.tensor_scalar(out=tmp[:], in0=posc[:],
                                                scalar1=float(j * P), scalar2=None,
                                                op0=ALU.is_ge)
                        nc.vector.tensor_add(posg[:], posg[:], tmp[:])
                    posp = rscr.tile([P, G], FP32, tag="posp")
                    nc.vector.tensor_scalar(out=posp[:], in0=posg[:],
                                            scalar1=float(-P), scalar2=None,
                                            op0=ALU.mult)
                    nc.vector.tensor_add(posp[:], posp[:], posc[:])

                    # compact via factored one-hot matmuls:
                    # out[slotp, 0:8]=ids, [8:16]=cw, [16:24]=hit
                    pc = psC.tile([P, 3 * CG], FP32, space="PSUM", tag="pc")
                    for g in range(G):
                        selp = rscr.tile([P, P], FP32, tag="selp")
                        nc.vector.tensor_scalar(out=selp[:], in0=iota128[:],
                                                scalar1=posp[:, g:g + 1],
                                                scalar2=None, op0=ALU.is_equal)
                        selg = rscr.tile([P, CG], FP32, tag="selg")
                        nc.vector.tensor_scalar(out=selg[:], in0=iotaCG[:],
                                                scalar1=posg[:, g:g + 1],
                                                scalar2=None, op0=ALU.is_equal)
                        rhs = rscr.tile([P, 3 * CG], FP32, tag="rhs")
                        nc.vector.tensor_scalar_mul(rhs[:, 0:CG], selg[:],
                                                    ids_f[:, g:g + 1])
                        nc.vector.tensor_scalar_mul(rhs[:, CG:2 * CG], selg[:],
                                                    cw_s[:, g:g + 1])
                        nc.vector.tensor_copy(out=rhs[:, 2 * CG:3 * CG], in_=selg[:])
                        nc.tensor.matmul(out=pc[:], lhsT=selp[:], rhs=rhs[:],
                                         start=(g == 0), stop=(g == G - 1))
                    # finalize: idx = ids + (1-hit)*N  (empty slots -> trash row)
                    idxf = rscr.tile([P, CG], FP32, tag="idxf")
                    nc.vector.tensor_scalar(out=idxf[:], in0=pc[:, 2 * CG:3 * CG],
                                            scalar1=float(-N), scalar2=float(N),
                                            op0=ALU.mult, op1=ALU.add)
                    nc.vector.tensor_add(idxf[:], idxf[:], pc[:, 0:CG])
                    nc.vector.tensor_copy(out=idx_sl[s][:], in_=idxf[:])
                    nc.vector.tensor_copy(out=cw_sl[s][:], in_=pc[:, CG:2 * CG])

            # ============ Phase B: per-slot gather + FFN + scatter-add ============
            if dbg_no_ffn:
                slot_list = []
            else:
                slot_list = [int(c) for c in dbg_slots]
            with contextlib.ExitStack() as pb:
                wp = pb.enter_context(tc.tile_pool(name="wp", bufs=20))
                xgp = pb.enter_context(tc.tile_pool(name="xgp", bufs=2))
                xtp = pb.enter_context(tc.tile_pool(name="xtp", bufs=2 * KD))
                hp = pb.enter_context(tc.tile_pool(name="hp", bufs=2 * MH))
                yp = pb.enter_context(tc.tile_pool(name="yp", bufs=2 * KD))
                ybp = pb.enter_context(tc.tile_pool(name="ybp", bufs=2))
                sscr = pb.enter_context(tc.tile_pool(name="sscr", bufs=3))
                ps1 = pb.enter_context(tc.tile_pool(name="ps1", bufs=2, space="PSUM"))
                ps2 = pb.enter_context(tc.tile_pool(name="ps2", bufs=2, space="PSUM"))
                psX = pb.enter_context(tc.tile_pool(name="psX", bufs=2, space="PSUM"))
                psB = pb.enter_context(tc.tile_pool(name="psB", bufs=2, space="PSUM"))

                for s in slot_list:
                    cap = CAPS[s]
                    if s < 2:
                        idx_t, cw_full = idx_sl[s], cw_sl[s]
                    else:
                        idx_t, cw_full = sh_idx, ones_cw
                    w1s = [wp.tile([P, H], BF, tag="w", name=f"w1s{s}_{k}")
                           for k in range(KD)]
                    for k in range(KD):
                        nc.sync.dma_start(out=w1s[k][:],
                                          in_=w1_d[s, k * P:(k + 1) * P, :])
                    w2s = [wp.tile([P, D], BF, tag="w", name=f"w2s{s}_{k}")
                           for k in range(MH)]
                    for k in range(MH):
                        nc.sync.dma_start(out=w2s[k][:],
                                          in_=w2_d[s, k * P:(k + 1) * P, :])
                    b1c = sscr.tile([P, MH], FP32, tag="b1c")
                    nc.sync.dma_start(out=b1c[:],
                                      in_=b1_d[s].rearrange("(m p) -> p m", p=P))
                    b2c = sscr.tile([P, KD], FP32, tag="b2c")
                    nc.sync.dma_start(out=b2c[:],
                                      in_=b2_d[s].rearrange("(m p) -> p m", p=P))

                    chunks = []
                    off = 0
                    while off < cap:
                        cl = min(512, cap - off)
                        chunks.append((off, cl))
                        off += cl
                    for (coff, clen) in chunks:
                        nk = clen // P
                        g0 = coff // P
                        xg = xgp.tile([P, nk * D], BF, tag="xg")
                        for g in range(nk):
                            nc.gpsimd.indirect_dma_start(
                                out=xg[:, g * D:(g + 1) * D], out_offset=None,
                                in_=xrows_d[:],
                                in_offset=bass.IndirectOffsetOnAxis(
                                    ap=idx_t[:, g0 + g:g0 + g + 1], axis=0))

                        # transpose gathered rows -> xgT chunks [P(d), clen]
                        xgT = [xtp.tile([P, clen], BF, tag="xgt", name=f"xgT{k}")
                               for k in range(KD)]
                        for g in range(nk):
                            for k in range(KD):
                                tp = psX.tile([P, P], BF, space="PSUM", tag="ptx")
                                nc.tensor.transpose(
                                    out=tp[:],
                                    in_=xg[:, g * D + k * P: g * D + (k + 1) * P],
                                    identity=ident_b[:])
                                nc.scalar.copy(out=xgT[k][:, g * P:(g + 1) * P],
                                               in_=tp[:])
                        # mm1 + relu -> h
                        hs = [hp.tile([P, clen], BF, tag="h", name=f"hs{m}")
                              for m in range(MH)]
                        for m in range(MH):
                            p1 = ps1.tile([P, clen], FP32, space="PSUM", tag="p1")
                            for k in range(KD):
                                nc.tensor.matmul(
                                    out=p1[:],
                                    lhsT=w1s[k][:, m * P:(m + 1) * P],
                                    rhs=xgT[k][:],
                                    start=(k == 0), stop=(k == KD - 1))
                            nc.scalar.activation(hs[m][:], p1[:], AF.Relu,
                                                 bias=b1c[:, m:m + 1], scale=1.0)
                        # mm2 + bias -> y (bf16)
                        ys = [yp.tile([P, clen], BF, tag="y", name=f"ys{d}")
                              for d in range(KD)]
                        for d in range(KD):
                            p2 = ps2.tile([P, clen], FP32, space="PSUM", tag="p2")
                            for k in range(MH):
                                nc.tensor.matmul(
                                    out=p2[:],
                                    lhsT=w2s[k][:, d * P:(d + 1) * P],
                                    rhs=hs[k][:],
                                    start=(k == 0), stop=(k == MH - 1))
                            nc.vector.tensor_scalar_add(ys[d][:], p2[:],
                                                        b2c[:, d:d + 1])
                        # transpose back + scale by cw -> yback f32, scatter-add
                        yb = ybp.tile([P, nk * D], FP32, tag="yb")
                        for g in range(nk):
                            for d in range(KD):
                                tb = psB.tile([P, P], BF, space="PSUM", tag="ptb")
                                nc.tensor.transpose(
                                    out=tb[:], in_=ys[d][:, g * P:(g + 1) * P],
                                    identity=ident_b[:])
                                nc.vector.tensor_scalar_mul(
                                    yb[:, g * D + d * P: g * D + (d + 1) * P],
                                    tb[:], cw_full[:, g0 + g:g0 + g + 1])
                        for g in range(nk):
                            nc.gpsimd.indirect_dma_start(
                                out=out_d[:],
                                out_offset=bass.IndirectOffsetOnAxis(
                                    ap=idx_t[:, g0 + g:g0 + g + 1], axis=0),
                                in_=yb[:, g * D:(g + 1) * D], in_offset=None,
                                compute_op=ALU.add)

    nc.compile()
    return nc


# expert-to-core assignment (pure host data; core 7 has 1 real expert)
EXPERT_PAIRS = [(2, 5), (4, 13), (12, 6), (3, 14), (8, 11), (1, 9), (0, 7), (10, -1)]

_CACHE = {}


def _get_program():
    if "nc" not in _CACHE:
        _CACHE["nc"] = build_program()
    return _CACHE["nc"]


def make_in_maps(x, router_W, router_b, sW1, sb1, sW2, sb2, eW1, eb1, eW2, eb2):
    xt = np.ascontiguousarray(x.reshape(N, D).T.astype(np.float32))
    xrows = np.zeros((N_PAD, D), BF16)
    xrows[:N] = x.reshape(N, D).astype(BF16)
    rW = np.ascontiguousarray(router_W.astype(np.float32))
    rb = np.ascontiguousarray(router_b.astype(np.float32).reshape(E, 1))
    in_maps = []
    for c in range(NCORES):
        e0, e1 = EXPERT_PAIRS[c]
        w1 = np.zeros((3, D, H), BF16)
        w2 = np.zeros((3, H, D), BF16)
        b1 = np.zeros((3, H), np.float32)
        b2 = np.zeros((3, D), np.float32)
        eidv = np.zeros((2, P), np.float32)
        for si, e in enumerate((e0, e1)):
            if e >= 0:
                w1[si] = eW1[e].astype(BF16)
                w2[si] = eW2[e].astype(BF16)
                b1[si] = eb1[e]
                b2[si] = eb2[e]
                eidv[si, :] = float(e)
            else:
                eidv[si, :] = -1.0
        w1[2] = sW1.astype(BF16)
        w2[2] = sW2.astype(BF16)
        b1[2] = sb1
        b2[2] = sb2
        shids = (np.arange(CAP_SH, dtype=np.int32) + c * CAP_SH).reshape(CAP_SH, 1)
        in_maps.append({
            "xT": xt, "xrows": xrows, "rW": rW, "rb": rb,
            "w1": w1, "w2": w2, "b1": b1, "b2": b2,
            "eid": eidv, "shids": shids,
        })
    return in_maps


def kernel(**inputs):
    from concourse import bass_utils
    nc = _get_program()
    in_maps = make_in_maps(**{k: np.asarray(v) for k, v in inputs.items()})
    res = bass_utils.run_bass_kernel_spmd(nc, in_maps, core_ids=list(range(NCORES)))
    out = np.zeros((N, D), np.float32)
    for r in res.results:
        out += r["out"][:N]
    return out.reshape(B, L, D)


if __name__ == "__main__":
    nc = _get_program()
    print("program built + compiled OK")
